# revision 2
# baseline (speedup 1.0000x reference)
"""LinkPredictor similarity kernel v2 for 8 Trainium2 NeuronCores.

reference:
    sims = E @ E.T               # [16384, 16384], E = [16384, 512] fp32
    m, M = sims.min(), sims.max()
    sims = (sims - m) / (M - m + 1e-7)
    out  = sims[row_idx, col_idx]     # block-diag strict-upper-tri gather

Key ideas vs the v1 baseline (fp32r sweep + min&max scans, 269 us sim):
  * Gram max is always on the diagonal: s_ij <= max(s_ii, s_jj) since
    ||x_i - x_j||^2 >= 0. M comes free from the exact diagonal-graph-block
    pass, eliminating the max scan entirely.
  * The min sweep only needs each entry to ~+-2 absolute out of a ~840
    range, so fp8 e4m3 inputs suffice (measured end-to-end rel err 0.7%
    vs the 2e-2 gate). fp8 DoubleRow contracts 2 k-tiles per pass at 0.5
    cycles/row: 4x PE throughput in the cost model, ~2x on silicon.
  * Min eviction splits ~0.72/0.28 between {ScalarE fp16 stage -> DVE fp16
    tensor_tensor min (2x mode)} and {DVE fp32 min direct from PSUM},
    keeping ScalarE and DVE busy in parallel under the PE. Two running
    buffers per class break serial dependency chains.
  * Diagonal slab-blocks skip quarter-tiles strictly below the diagonal
    (mirrors of already-covered upper-triangle entries).
  * The diagonal-graph-block pass is interleaved into the sweep so its
    DMA/PE/ScalarE work hides under the min scan.

Distribution: 16 half-slabs of 1024 rows. Core c owns half-slabs
{c, 15-c} and the 17 upper-triangle [1024,1024] blocks whose row
half-slab is one of those. Items are ordered [diag, diag, 15 x offdiag]
so one SPMD program serves all cores. Host combines min/max, normalizes,
gathers.
"""

import numpy as np

N_GRAPHS = 128
G = 128
D = 512
N = N_GRAPHS * G          # 16384
EPS = 1e-7
NCORES = 8
HS = 1024                 # half-slab rows
NHS = N // HS             # 16 half-slabs
NBLK = 17                 # triangle blocks per core
KC = D // 128             # 4 contraction k-tiles
MT = HS // 128            # 8 m-strips per block
GPC = 16                  # graphs per core

NRUN = 2                  # running-min buffers per class
STAGE_NUM, STAGE_DEN = 18, 25   # staged fraction of evictions

_CACHED = {}

# first needed column-quarter per m-strip inside a diagonal slab-block
_DIAG_Q0 = [0, 0, 1, 1, 2, 2, 3, 3]


def _build_program():
    import concourse.bacc as bacc
    import concourse.mybir as mybir
    from concourse.tile import TileContext

    f32 = mybir.dt.float32
    f16 = mybir.dt.float16
    f8 = mybir.dt.float8e4
    DR = mybir.MatmulPerfMode.DoubleRow
    MIN = mybir.AluOpType.min

    nc = bacc.Bacc(target_bir_lowering=False)
    lhs8 = nc.declare_dram_parameter("lhs8", [NBLK, 128, KC, HS], f8, isOutput=False)
    rhs8 = nc.declare_dram_parameter("rhs8", [NBLK, 128, KC, HS], f8, isOutput=False)
    dg16 = nc.declare_dram_parameter("dg16", [GPC, 128, KC, G], f16, isOutput=False)
    diag_out = nc.declare_dram_parameter("diag_out", [GPC, G, G], f32, isOutput=True)
    min_out = nc.declare_dram_parameter(
        "min_out", [128, 2 * NRUN], f32, isOutput=True
    )

    with TileContext(nc) as tc:
        with (
            tc.tile_pool(name="persist", bufs=1) as persist,
            tc.tile_pool(name="stream", bufs=3) as stream,
            tc.tile_pool(name="staged", bufs=4) as staged,
            tc.tile_pool(name="small", bufs=4) as small,
            tc.tile_pool(name="ps", bufs=3, space="PSUM") as ps,
            tc.tile_pool(name="psd", bufs=2, space="PSUM") as psd,
        ):
            run16 = [
                persist.tile([128, HS], f16, tag=f"run16_{i}", name=f"run16_{i}")
                for i in range(NRUN)
            ]
            run32 = [
                persist.tile([128, HS], f32, tag=f"run32_{i}", name=f"run32_{i}")
                for i in range(NRUN)
            ]
            for r in run16:
                nc.vector.memset(r[:], 60000.0)
            for r in run32:
                nc.vector.memset(r[:], 3.0e38)

            def emit_dg(g):
                dgt = small.tile([128, KC, G], f16, tag="dgt", name="dgt")
                nc.sync.dma_start(out=dgt[:], in_=dg16[g])
                dacc = psd.tile([128, G], f32, tag="dacc", name="dacc")
                for k in range(KC):
                    nc.tensor.matmul(
                        dacc[:], dgt[:, k, :], dgt[:, k, :],
                        start=(k == 0), stop=(k == KC - 1),
                    )
                dcp = small.tile([128, G], f32, tag="dcp", name="dcp")
                nc.scalar.copy(dcp[:], dacc[:])
                nc.sync.dma_start(out=diag_out[g], in_=dcp[:])

            tile_counter = 0
            n_staged = 0
            n_direct = 0
            for item in range(NBLK):
                if item < GPC:
                    emit_dg(item)
                is_diag = item < 2
                rt = stream.tile([128, KC, HS], f8, tag="rt", name="rt")
                nc.sync.dma_start(out=rt[:], in_=rhs8[item])
                if is_diag:
                    lt = rt
                else:
                    lt = stream.tile([128, KC, HS], f8, tag="lt", name="lt")
                    nc.sync.dma_start(out=lt[:], in_=lhs8[item])
                for m in range(MT):
                    q0 = _DIAG_Q0[m] if is_diag else 0
                    width = (4 - q0) * 256
                    sup = ps.tile([128, HS], f32, tag="sup", name="sup")
                    for q in range(q0, 4):
                        for p in range(2):
                            nc.tensor.matmul(
                                sup[:, q * 256 : (q + 1) * 256],
                                lt[:, 2 * p : 2 * p + 2, m * 128 : (m + 1) * 128],
                                rt[:, 2 * p : 2 * p + 2, q * 256 : (q + 1) * 256],
                                start=(p == 0), stop=(p == 1),
                                perf_mode=DR,
                            )
                    src = sup[:, q0 * 256 : HS]
                    stagedp = (tile_counter * STAGE_NUM) // STAGE_DEN != (
                        (tile_counter + 1) * STAGE_NUM
                    ) // STAGE_DEN
                    tile_counter += 1
                    if stagedp:
                        st = staged.tile([128, HS], f16, tag="st", name="st")
                        nc.scalar.copy(st[:, :width], src)
                        r = run16[n_staged % NRUN]
                        n_staged += 1
                        nc.vector.tensor_tensor(
                            r[:, :width], r[:, :width], st[:, :width], MIN
                        )
                    else:
                        r = run32[n_direct % NRUN]
                        n_direct += 1
                        nc.vector.tensor_tensor(
                            r[:, :width], r[:, :width], src, MIN
                        )

            # tree-fold the four running buffers, then one reduce
            nc.vector.tensor_tensor(run32[0][:], run32[0][:], run32[1][:], MIN)
            nc.vector.tensor_tensor(run16[0][:], run16[0][:], run16[1][:], MIN)
            nc.vector.tensor_tensor(run16[0][:], run16[0][:], run32[0][:], MIN)
            red = small.tile([128, 2 * NRUN], f32, tag="red", name="red")
            nc.vector.memset(red[:], 3.0e38)
            nc.vector.tensor_reduce(
                red[:, 0:1], run16[0][:], mybir.AxisListType.X, MIN
            )
            nc.sync.dma_start(out=min_out[:], in_=red[:])

    nc.finalize()
    return nc


def _core_rows(c: int):
    return [c, NHS - 1 - c]


def _core_items(c: int):
    i0, i1 = _core_rows(c)
    items = [(i0, i0), (i1, i1)]
    items += [(i0, j) for j in range(i0 + 1, NHS)]
    items += [(i1, j) for j in range(i1 + 1, NHS)]
    assert len(items) == NBLK
    return items


def _core_graphs(c: int):
    gph = HS // G  # 8 graphs per half-slab
    out = []
    for i in _core_rows(c):
        out.extend(range(i * gph, i * gph + gph))
    return out


def build_in_maps(emb: np.ndarray):
    import ml_dtypes

    emb = np.asarray(emb, dtype=np.float32)
    e8 = emb.astype(ml_dtypes.float8_e4m3)
    e16 = emb.astype(np.float16)

    # packed [p, t, col]: element = E[col, t*128 + p]
    p8 = np.ascontiguousarray(e8.T.reshape(KC, 128, N).transpose(1, 0, 2))
    p16 = np.ascontiguousarray(e16.T.reshape(KC, 128, N).transpose(1, 0, 2))
    slab8 = [
        np.ascontiguousarray(p8[:, :, s * HS : (s + 1) * HS]) for s in range(NHS)
    ]
    graph16 = [
        np.ascontiguousarray(p16[:, :, g * G : (g + 1) * G])
        for g in range(N_GRAPHS)
    ]

    in_maps = []
    for c in range(NCORES):
        items = _core_items(c)
        lhs = np.stack([slab8[i] for i, _ in items])
        rhs = np.stack([slab8[j] for _, j in items])
        dg = np.stack([graph16[g] for g in _core_graphs(c)])
        in_maps.append({"lhs8": lhs, "rhs8": rhs, "dg16": dg})
    return in_maps


def kernel(embeddings, row_idx, col_idx):
    from concourse.bass_utils import run_bass_kernel_spmd

    emb = np.asarray(embeddings, dtype=np.float32)
    row_idx = np.asarray(row_idx)
    col_idx = np.asarray(col_idx)

    if "nc" not in _CACHED:
        _CACHED["nc"] = _build_program()
    nc = _CACHED["nc"]

    in_maps = build_in_maps(emb)
    res = run_bass_kernel_spmd(nc, in_maps, list(range(NCORES)))

    m = min(float(r["min_out"].min()) for r in res.results)

    blocks = np.empty((N_GRAPHS, G, G), np.float32)
    for c in range(NCORES):
        for idx, g in enumerate(_core_graphs(c)):
            blocks[g] = res.results[c]["diag_out"][idx]

    # Gram max is attained on the diagonal (AM-GM), which the exact
    # diagonal blocks contain.
    M = float(blocks[:, np.arange(G), np.arange(G)].max())

    norm = (blocks - m) / (M - m + EPS)
    blk = row_idx // G
    out = norm[blk, row_idx % G, col_idx % G].astype(np.float32)
    return out


# revision 4
# speedup vs baseline: 2070.0029x; 2070.0029x over previous
"""LinkPredictor similarity kernel v2 for 8 Trainium2 NeuronCores.

reference:
    sims = E @ E.T               # [16384, 16384], E = [16384, 512] fp32
    m, M = sims.min(), sims.max()
    sims = (sims - m) / (M - m + 1e-7)
    out  = sims[row_idx, col_idx]     # block-diag strict-upper-tri gather

Key ideas vs the v1 baseline (fp32r sweep + min&max scans, 269 us sim):
  * Gram max is always on the diagonal: s_ij <= max(s_ii, s_jj) since
    ||x_i - x_j||^2 >= 0. M comes free from the exact diagonal-graph-block
    pass, eliminating the max scan entirely.
  * The min sweep only needs each entry to ~+-2 absolute out of a ~840
    range, so fp8 e4m3 inputs suffice (measured end-to-end rel err 0.7%
    vs the 2e-2 gate). fp8 DoubleRow contracts 2 k-tiles per pass at 0.5
    cycles/row: 4x PE throughput in the cost model, ~2x on silicon.
  * Min eviction splits ~0.72/0.28 between {ScalarE fp16 stage -> DVE fp16
    tensor_tensor min (2x mode)} and {DVE fp32 min direct from PSUM},
    keeping ScalarE and DVE busy in parallel under the PE. Two running
    buffers per class break serial dependency chains.
  * Diagonal slab-blocks skip quarter-tiles strictly below the diagonal
    (mirrors of already-covered upper-triangle entries).
  * The diagonal-graph-block pass is interleaved into the sweep so its
    DMA/PE/ScalarE work hides under the min scan.

Distribution: 16 half-slabs of 1024 rows. Core c owns half-slabs
{c, 15-c} and the 17 upper-triangle [1024,1024] blocks whose row
half-slab is one of those. Items are ordered [diag, diag, 15 x offdiag]
so one SPMD program serves all cores. Host combines min/max, normalizes,
gathers.
"""

import numpy as np

N_GRAPHS = 128
G = 128
D = 512
N = N_GRAPHS * G          # 16384
EPS = 1e-7
NCORES = 8
HS = 1024                 # half-slab rows
NHS = N // HS             # 16 half-slabs
NBLK = 17                 # triangle blocks per core
KC = D // 128             # 4 contraction k-tiles
MT = HS // 128            # 8 m-strips per block
GPC = 16                  # graphs per core

NRUN = 2                  # running-min buffers per class
STAGE_NUM, STAGE_DEN = 18, 25   # staged fraction of evictions

_CACHED = {}

# first needed column-quarter per m-strip inside a diagonal slab-block
_DIAG_Q0 = [0, 0, 1, 1, 2, 2, 3, 3]


def _build_program():
    import concourse.bacc as bacc
    import concourse.mybir as mybir
    from concourse.tile import TileContext

    f32 = mybir.dt.float32
    f16 = mybir.dt.float16
    f8 = mybir.dt.float8e4
    DR = mybir.MatmulPerfMode.DoubleRow
    MIN = mybir.AluOpType.min

    nc = bacc.Bacc(target_bir_lowering=False)
    lhs8 = nc.declare_dram_parameter("lhs8", [NBLK, 128, KC, HS], f8, isOutput=False)
    rhs8 = nc.declare_dram_parameter("rhs8", [NBLK, 128, KC, HS], f8, isOutput=False)
    dg16 = nc.declare_dram_parameter("dg16", [GPC, 128, KC, G], f16, isOutput=False)
    diag_out = nc.declare_dram_parameter("diag_out", [GPC, G, G], f32, isOutput=True)
    min_out = nc.declare_dram_parameter(
        "min_out", [128, 2 * NRUN], f32, isOutput=True
    )

    with TileContext(nc) as tc:
        with (
            tc.tile_pool(name="persist", bufs=1) as persist,
            tc.tile_pool(name="stream", bufs=3) as stream,
            tc.tile_pool(name="staged", bufs=4) as staged,
            tc.tile_pool(name="small", bufs=4) as small,
            tc.tile_pool(name="ps", bufs=3, space="PSUM") as ps,
            tc.tile_pool(name="psd", bufs=2, space="PSUM") as psd,
        ):
            run16 = [
                persist.tile([128, HS], f16, tag=f"run16_{i}", name=f"run16_{i}")
                for i in range(NRUN)
            ]
            run32 = [
                persist.tile([128, HS], f32, tag=f"run32_{i}", name=f"run32_{i}")
                for i in range(NRUN)
            ]
            for r in run16:
                nc.vector.memset(r[:], 60000.0)
            for r in run32:
                nc.vector.memset(r[:], 3.0e38)

            def emit_dg(g):
                dgt = small.tile([128, KC, G], f16, tag="dgt", name="dgt")
                nc.sync.dma_start(out=dgt[:], in_=dg16[g])
                dacc = psd.tile([128, G], f32, tag="dacc", name="dacc")
                for k in range(KC):
                    nc.tensor.matmul(
                        dacc[:], dgt[:, k, :], dgt[:, k, :],
                        start=(k == 0), stop=(k == KC - 1),
                    )
                dcp = small.tile([128, G], f32, tag="dcp", name="dcp")
                nc.scalar.copy(dcp[:], dacc[:])
                nc.sync.dma_start(out=diag_out[g], in_=dcp[:])

            tile_counter = 0
            n_staged = 0
            n_direct = 0
            for item in range(NBLK):
                if item < GPC:
                    emit_dg(item)
                is_diag = item < 2
                rt = stream.tile([128, KC, HS], f8, tag="rt", name="rt")
                nc.sync.dma_start(out=rt[:], in_=rhs8[item])
                if is_diag:
                    lt = rt
                else:
                    lt = stream.tile([128, KC, HS], f8, tag="lt", name="lt")
                    nc.sync.dma_start(out=lt[:], in_=lhs8[item])
                for m in range(MT):
                    q0 = _DIAG_Q0[m] if is_diag else 0
                    width = (4 - q0) * 256
                    sup = ps.tile([128, HS], f32, tag="sup", name="sup")
                    for q in range(q0, 4):
                        for p in range(2):
                            nc.tensor.matmul(
                                sup[:, q * 256 : (q + 1) * 256],
                                lt[:, 2 * p : 2 * p + 2, m * 128 : (m + 1) * 128],
                                rt[:, 2 * p : 2 * p + 2, q * 256 : (q + 1) * 256],
                                start=(p == 0), stop=(p == 1),
                                perf_mode=DR,
                            )
                    src = sup[:, q0 * 256 : HS]
                    stagedp = (tile_counter * STAGE_NUM) // STAGE_DEN != (
                        (tile_counter + 1) * STAGE_NUM
                    ) // STAGE_DEN
                    tile_counter += 1
                    if stagedp:
                        st = staged.tile([128, HS], f16, tag="st", name="st")
                        nc.scalar.copy(st[:, :width], src)
                        r = run16[n_staged % NRUN]
                        n_staged += 1
                        nc.vector.tensor_tensor(
                            r[:, :width], r[:, :width], st[:, :width], MIN
                        )
                    else:
                        r = run32[n_direct % NRUN]
                        n_direct += 1
                        nc.vector.tensor_tensor(
                            r[:, :width], r[:, :width], src, MIN
                        )

            # tree-fold the running buffers, then one reduce
            nc.vector.tensor_tensor(run32[0][:], run32[0][:], run32[1][:], MIN)
            nc.vector.tensor_tensor(run16[0][:], run16[0][:], run16[1][:], MIN)
            nc.vector.tensor_tensor(run16[0][:], run16[0][:], run32[0][:], MIN)
            red = small.tile([128, 2 * NRUN], f32, tag="red", name="red")
            nc.vector.memset(red[:], 3.0e38)
            nc.vector.tensor_reduce(
                red[:, 0:1], run16[0][:], mybir.AxisListType.X, MIN
            )
            nc.sync.dma_start(out=min_out[:], in_=red[:])

    nc.finalize()
    return nc


def _core_rows(c: int):
    return [c, NHS - 1 - c]


def _core_items(c: int):
    i0, i1 = _core_rows(c)
    items = [(i0, i0), (i1, i1)]
    items += [(i0, j) for j in range(i0 + 1, NHS)]
    items += [(i1, j) for j in range(i1 + 1, NHS)]
    assert len(items) == NBLK
    return items


def _core_graphs(c: int):
    gph = HS // G  # 8 graphs per half-slab
    out = []
    for i in _core_rows(c):
        out.extend(range(i * gph, i * gph + gph))
    return out


def build_in_maps(emb: np.ndarray):
    import ml_dtypes

    emb = np.asarray(emb, dtype=np.float32)
    e8 = emb.astype(ml_dtypes.float8_e4m3)
    e16 = emb.astype(np.float16)

    # packed [p, t, col]: element = E[col, t*128 + p]
    p8 = np.ascontiguousarray(e8.T.reshape(KC, 128, N).transpose(1, 0, 2))
    p16 = np.ascontiguousarray(e16.T.reshape(KC, 128, N).transpose(1, 0, 2))
    slab8 = [
        np.ascontiguousarray(p8[:, :, s * HS : (s + 1) * HS]) for s in range(NHS)
    ]
    graph16 = [
        np.ascontiguousarray(p16[:, :, g * G : (g + 1) * G])
        for g in range(N_GRAPHS)
    ]

    in_maps = []
    for c in range(NCORES):
        items = _core_items(c)
        lhs = np.stack([slab8[i] for i, _ in items])
        rhs = np.stack([slab8[j] for _, j in items])
        dg = np.stack([graph16[g] for g in _core_graphs(c)])
        in_maps.append({"lhs8": lhs, "rhs8": rhs, "dg16": dg})
    return in_maps


def kernel(embeddings, row_idx, col_idx):
    from concourse.bass_utils import run_bass_kernel_spmd

    emb = np.asarray(embeddings, dtype=np.float32)
    row_idx = np.asarray(row_idx)
    col_idx = np.asarray(col_idx)

    if "nc" not in _CACHED:
        _CACHED["nc"] = _build_program()
    nc = _CACHED["nc"]

    in_maps = build_in_maps(emb)
    res = run_bass_kernel_spmd(nc, in_maps, list(range(NCORES)))

    m = min(float(r["min_out"].min()) for r in res.results)

    blocks = np.empty((N_GRAPHS, G, G), np.float32)
    for c in range(NCORES):
        for idx, g in enumerate(_core_graphs(c)):
            blocks[g] = res.results[c]["diag_out"][idx]

    # Gram max is attained on the diagonal (AM-GM), which the exact
    # diagonal blocks contain.
    M = float(blocks[:, np.arange(G), np.arange(G)].max())

    norm = (blocks - m) / (M - m + EPS)
    blk = row_idx // G
    out = norm[blk, row_idx % G, col_idx % G].astype(np.float32)
    return out


# revision 5
# speedup vs baseline: 2089.4082x; 1.0094x over previous
"""LinkPredictor similarity kernel v2 for 8 Trainium2 NeuronCores.

reference:
    sims = E @ E.T               # [16384, 16384], E = [16384, 512] fp32
    m, M = sims.min(), sims.max()
    sims = (sims - m) / (M - m + 1e-7)
    out  = sims[row_idx, col_idx]     # block-diag strict-upper-tri gather

Key ideas vs the v1 baseline (fp32r sweep + min&max scans, 269 us sim):
  * Gram max is always on the diagonal: s_ij <= max(s_ii, s_jj) since
    ||x_i - x_j||^2 >= 0. M comes free from the exact diagonal-graph-block
    pass, eliminating the max scan entirely.
  * The min sweep only needs each entry to ~+-2 absolute out of a ~840
    range, so fp8 e4m3 inputs suffice (measured end-to-end rel err 0.7%
    vs the 2e-2 gate). fp8 DoubleRow contracts 2 k-tiles per pass at 0.5
    cycles/row: 4x PE throughput in the cost model, ~2x on silicon.
  * Min eviction splits ~0.72/0.28 between {ScalarE fp16 stage -> DVE fp16
    tensor_tensor min (2x mode)} and {DVE fp32 min direct from PSUM},
    keeping ScalarE and DVE busy in parallel under the PE. Two running
    buffers per class break serial dependency chains.
  * Diagonal slab-blocks skip quarter-tiles strictly below the diagonal
    (mirrors of already-covered upper-triangle entries).
  * The diagonal-graph-block pass is interleaved into the sweep so its
    DMA/PE/ScalarE work hides under the min scan.

Distribution: 16 half-slabs of 1024 rows. Core c owns half-slabs
{c, 15-c} and the 17 upper-triangle [1024,1024] blocks whose row
half-slab is one of those. Items are ordered [diag, diag, 15 x offdiag]
so one SPMD program serves all cores. Host combines min/max, normalizes,
gathers.
"""

import numpy as np

N_GRAPHS = 128
G = 128
D = 512
N = N_GRAPHS * G          # 16384
EPS = 1e-7
NCORES = 8
HS = 1024                 # half-slab rows
NHS = N // HS             # 16 half-slabs
NBLK = 17                 # triangle blocks per core
KC = D // 128             # 4 contraction k-tiles
MT = HS // 128            # 8 m-strips per block
GPC = 16                  # graphs per core

NRUN = 2                  # running-min buffers per class
STAGE_NUM, STAGE_DEN = 5, 7     # staged fraction of evictions

_CACHED = {}

# first needed column-quarter per m-strip inside a diagonal slab-block
_DIAG_Q0 = [0, 0, 1, 1, 2, 2, 3, 3]


def _build_program():
    import concourse.bacc as bacc
    import concourse.mybir as mybir
    from concourse.tile import TileContext

    f32 = mybir.dt.float32
    f16 = mybir.dt.float16
    f8 = mybir.dt.float8e4
    DR = mybir.MatmulPerfMode.DoubleRow
    MIN = mybir.AluOpType.min

    nc = bacc.Bacc(target_bir_lowering=False)
    lhs8 = nc.declare_dram_parameter("lhs8", [NBLK, 128, KC, HS], f8, isOutput=False)
    rhs8 = nc.declare_dram_parameter("rhs8", [NBLK, 128, KC, HS], f8, isOutput=False)
    dg16 = nc.declare_dram_parameter("dg16", [GPC, 128, KC, G], f16, isOutput=False)
    diag_out = nc.declare_dram_parameter("diag_out", [GPC, G, G], f32, isOutput=True)
    min_out = nc.declare_dram_parameter(
        "min_out", [128, 2 * NRUN], f32, isOutput=True
    )

    with TileContext(nc) as tc:
        with (
            tc.tile_pool(name="persist", bufs=1) as persist,
            tc.tile_pool(name="stream", bufs=3) as stream,
            tc.tile_pool(name="staged", bufs=4) as staged,
            tc.tile_pool(name="small", bufs=4) as small,
            tc.tile_pool(name="ps", bufs=3, space="PSUM") as ps,
            tc.tile_pool(name="psd", bufs=2, space="PSUM") as psd,
        ):
            run16 = [
                persist.tile([128, HS], f16, tag=f"run16_{i}", name=f"run16_{i}")
                for i in range(NRUN)
            ]
            run32 = [
                persist.tile([128, HS], f32, tag=f"run32_{i}", name=f"run32_{i}")
                for i in range(NRUN)
            ]
            for r in run16:
                nc.vector.memset(r[:], 60000.0)
            for r in run32:
                nc.vector.memset(r[:], 3.0e38)

            def emit_dg(g):
                dgt = small.tile([128, KC, G], f16, tag="dgt", name="dgt")
                nc.sync.dma_start(out=dgt[:], in_=dg16[g])
                dacc = psd.tile([128, G], f32, tag="dacc", name="dacc")
                for k in range(KC):
                    nc.tensor.matmul(
                        dacc[:], dgt[:, k, :], dgt[:, k, :],
                        start=(k == 0), stop=(k == KC - 1),
                    )
                dcp = small.tile([128, G], f32, tag="dcp", name="dcp")
                nc.scalar.copy(dcp[:], dacc[:])
                nc.sync.dma_start(out=diag_out[g], in_=dcp[:])

            tile_counter = 0
            n_staged = 0
            n_direct = 0
            for item in range(NBLK):
                if item < GPC:
                    emit_dg(item)
                is_diag = item < 2
                rt = stream.tile([128, KC, HS], f8, tag="rt", name="rt")
                nc.sync.dma_start(out=rt[:], in_=rhs8[item])
                if is_diag:
                    lt = rt
                else:
                    lt = stream.tile([128, KC, HS], f8, tag="lt", name="lt")
                    nc.sync.dma_start(out=lt[:], in_=lhs8[item])
                for m in range(MT):
                    q0 = _DIAG_Q0[m] if is_diag else 0
                    width = (4 - q0) * 256
                    sup = ps.tile([128, HS], f32, tag="sup", name="sup")
                    for q in range(q0, 4):
                        for p in range(2):
                            nc.tensor.matmul(
                                sup[:, q * 256 : (q + 1) * 256],
                                lt[:, 2 * p : 2 * p + 2, m * 128 : (m + 1) * 128],
                                rt[:, 2 * p : 2 * p + 2, q * 256 : (q + 1) * 256],
                                start=(p == 0), stop=(p == 1),
                                perf_mode=DR,
                            )
                    src = sup[:, q0 * 256 : HS]
                    stagedp = (tile_counter * STAGE_NUM) // STAGE_DEN != (
                        (tile_counter + 1) * STAGE_NUM
                    ) // STAGE_DEN
                    tile_counter += 1
                    if stagedp:
                        st = staged.tile([128, HS], f16, tag="st", name="st")
                        nc.scalar.copy(st[:, :width], src)
                        r = run16[n_staged % NRUN]
                        n_staged += 1
                        nc.vector.tensor_tensor(
                            r[:, :width], r[:, :width], st[:, :width], MIN
                        )
                    else:
                        r = run32[n_direct % NRUN]
                        n_direct += 1
                        nc.vector.tensor_tensor(
                            r[:, :width], r[:, :width], src, MIN
                        )

            # tree-fold the running buffers, then one reduce
            nc.vector.tensor_tensor(run32[0][:], run32[0][:], run32[1][:], MIN)
            nc.vector.tensor_tensor(run16[0][:], run16[0][:], run16[1][:], MIN)
            nc.vector.tensor_tensor(run16[0][:], run16[0][:], run32[0][:], MIN)
            red = small.tile([128, 2 * NRUN], f32, tag="red", name="red")
            nc.vector.memset(red[:], 3.0e38)
            nc.vector.tensor_reduce(
                red[:, 0:1], run16[0][:], mybir.AxisListType.X, MIN
            )
            nc.sync.dma_start(out=min_out[:], in_=red[:])

    nc.finalize()
    return nc


def _core_rows(c: int):
    return [c, NHS - 1 - c]


def _core_items(c: int):
    i0, i1 = _core_rows(c)
    items = [(i0, i0), (i1, i1)]
    items += [(i0, j) for j in range(i0 + 1, NHS)]
    items += [(i1, j) for j in range(i1 + 1, NHS)]
    assert len(items) == NBLK
    return items


def _core_graphs(c: int):
    gph = HS // G  # 8 graphs per half-slab
    out = []
    for i in _core_rows(c):
        out.extend(range(i * gph, i * gph + gph))
    return out


def build_in_maps(emb: np.ndarray):
    import ml_dtypes

    emb = np.asarray(emb, dtype=np.float32)
    e8 = emb.astype(ml_dtypes.float8_e4m3)
    e16 = emb.astype(np.float16)

    # packed [p, t, col]: element = E[col, t*128 + p]
    p8 = np.ascontiguousarray(e8.T.reshape(KC, 128, N).transpose(1, 0, 2))
    p16 = np.ascontiguousarray(e16.T.reshape(KC, 128, N).transpose(1, 0, 2))
    slab8 = [
        np.ascontiguousarray(p8[:, :, s * HS : (s + 1) * HS]) for s in range(NHS)
    ]
    graph16 = [
        np.ascontiguousarray(p16[:, :, g * G : (g + 1) * G])
        for g in range(N_GRAPHS)
    ]

    in_maps = []
    for c in range(NCORES):
        items = _core_items(c)
        lhs = np.stack([slab8[i] for i, _ in items])
        rhs = np.stack([slab8[j] for _, j in items])
        dg = np.stack([graph16[g] for g in _core_graphs(c)])
        in_maps.append({"lhs8": lhs, "rhs8": rhs, "dg16": dg})
    return in_maps


def kernel(embeddings, row_idx, col_idx):
    from concourse.bass_utils import run_bass_kernel_spmd

    emb = np.asarray(embeddings, dtype=np.float32)
    row_idx = np.asarray(row_idx)
    col_idx = np.asarray(col_idx)

    if "nc" not in _CACHED:
        _CACHED["nc"] = _build_program()
    nc = _CACHED["nc"]

    in_maps = build_in_maps(emb)
    res = run_bass_kernel_spmd(nc, in_maps, list(range(NCORES)))

    m = min(float(r["min_out"].min()) for r in res.results)

    blocks = np.empty((N_GRAPHS, G, G), np.float32)
    for c in range(NCORES):
        for idx, g in enumerate(_core_graphs(c)):
            blocks[g] = res.results[c]["diag_out"][idx]

    # Gram max is attained on the diagonal (AM-GM), which the exact
    # diagonal blocks contain.
    M = float(blocks[:, np.arange(G), np.arange(G)].max())

    norm = (blocks - m) / (M - m + EPS)
    blk = row_idx // G
    out = norm[blk, row_idx % G, col_idx % G].astype(np.float32)
    return out


# revision 6
# speedup vs baseline: 2109.9353x; 1.0098x over previous
"""LinkPredictor similarity kernel v2 for 8 Trainium2 NeuronCores.

reference:
    sims = E @ E.T               # [16384, 16384], E = [16384, 512] fp32
    m, M = sims.min(), sims.max()
    sims = (sims - m) / (M - m + 1e-7)
    out  = sims[row_idx, col_idx]     # block-diag strict-upper-tri gather

Key ideas vs the v1 baseline (fp32r sweep + min&max scans, 269 us sim):
  * Gram max is always on the diagonal: s_ij <= max(s_ii, s_jj) since
    ||x_i - x_j||^2 >= 0. M comes free from the exact diagonal-graph-block
    pass, eliminating the max scan entirely.
  * The min sweep only needs each entry to ~+-2 absolute out of a ~840
    range, so fp8 e4m3 inputs suffice (measured end-to-end rel err 0.7%
    vs the 2e-2 gate). fp8 DoubleRow contracts 2 k-tiles per pass at 0.5
    cycles/row: 4x PE throughput in the cost model, ~2x on silicon.
  * Min eviction splits ~0.72/0.28 between {ScalarE fp16 stage -> DVE fp16
    tensor_tensor min (2x mode)} and {DVE fp32 min direct from PSUM},
    keeping ScalarE and DVE busy in parallel under the PE. Two running
    buffers per class break serial dependency chains.
  * Diagonal slab-blocks skip quarter-tiles strictly below the diagonal
    (mirrors of already-covered upper-triangle entries).
  * The diagonal-graph-block pass is interleaved into the sweep so its
    DMA/PE/ScalarE work hides under the min scan.

Distribution: 16 half-slabs of 1024 rows. Core c owns half-slabs
{c, 15-c} and the 17 upper-triangle [1024,1024] blocks whose row
half-slab is one of those. Items are ordered [diag, diag, 15 x offdiag]
so one SPMD program serves all cores. Host combines min/max, normalizes,
gathers.
"""

import numpy as np

N_GRAPHS = 128
G = 128
D = 512
N = N_GRAPHS * G          # 16384
EPS = 1e-7
NCORES = 8
HS = 1024                 # half-slab rows
NHS = N // HS             # 16 half-slabs
NBLK = 17                 # triangle blocks per core
KC = D // 128             # 4 contraction k-tiles
MT = HS // 128            # 8 m-strips per block
GPC = 16                  # graphs per core

NRUN = 2                  # running-min buffers per class
STAGE_NUM, STAGE_DEN = 5, 7     # staged fraction of evictions

_CACHED = {}

# first needed column-quarter per m-strip inside a diagonal slab-block
_DIAG_Q0 = [0, 0, 1, 1, 2, 2, 3, 3]


def _build_program():
    import concourse.bacc as bacc
    import concourse.mybir as mybir
    from concourse.tile import TileContext

    f32 = mybir.dt.float32
    f16 = mybir.dt.float16
    f8 = mybir.dt.float8e4
    DR = mybir.MatmulPerfMode.DoubleRow
    MIN = mybir.AluOpType.min

    nc = bacc.Bacc(target_bir_lowering=False)
    lhs8 = nc.declare_dram_parameter("lhs8", [NBLK, 128, KC, HS], f8, isOutput=False)
    rhs8 = nc.declare_dram_parameter("rhs8", [NBLK, 128, KC, HS], f8, isOutput=False)
    dg16 = nc.declare_dram_parameter("dg16", [GPC, 128, KC, G], f16, isOutput=False)
    diag_out = nc.declare_dram_parameter("diag_out", [GPC, G, G], f32, isOutput=True)
    min_out = nc.declare_dram_parameter(
        "min_out", [128, 2 * NRUN], f32, isOutput=True
    )

    with TileContext(nc) as tc:
        with (
            tc.tile_pool(name="persist", bufs=1) as persist,
            tc.tile_pool(name="stream", bufs=3) as stream,
            tc.tile_pool(name="staged", bufs=4) as staged,
            tc.tile_pool(name="small", bufs=4) as small,
            tc.tile_pool(name="ps", bufs=4, space="PSUM") as ps,
        ):
            run16 = [
                persist.tile([128, HS], f16, tag=f"run16_{i}", name=f"run16_{i}")
                for i in range(NRUN)
            ]
            run32 = [
                persist.tile([128, HS], f32, tag=f"run32_{i}", name=f"run32_{i}")
                for i in range(NRUN)
            ]
            for r in run16:
                nc.vector.memset(r[:], 60000.0)
            for r in run32:
                nc.vector.memset(r[:], 3.0e38)

            def emit_dg(g):
                dgt = small.tile([128, KC, G], f16, tag="dgt", name="dgt")
                nc.sync.dma_start(out=dgt[:], in_=dg16[g])
                dacc_t = ps.tile([128, HS], f32, tag="sup", name="dacc_sup")
                dacc = dacc_t[:, :G]
                for k in range(KC):
                    nc.tensor.matmul(
                        dacc, dgt[:, k, :], dgt[:, k, :],
                        start=(k == 0), stop=(k == KC - 1),
                    )
                dcp = small.tile([128, G], f32, tag="dcp", name="dcp")
                nc.scalar.copy(dcp[:], dacc)
                nc.sync.dma_start(out=diag_out[g], in_=dcp[:])

            tile_counter = 0
            n_staged = 0
            n_direct = 0
            for item in range(NBLK):
                if item < GPC:
                    emit_dg(item)
                is_diag = item < 2
                rt = stream.tile([128, KC, HS], f8, tag="rt", name="rt")
                nc.sync.dma_start(out=rt[:], in_=rhs8[item])
                if is_diag:
                    lt = rt
                else:
                    lt = stream.tile([128, KC, HS], f8, tag="lt", name="lt")
                    nc.sync.dma_start(out=lt[:], in_=lhs8[item])
                for m in range(MT):
                    q0 = _DIAG_Q0[m] if is_diag else 0
                    lo = m * 128 if is_diag else 0
                    width = HS - lo
                    sup = ps.tile([128, HS], f32, tag="sup", name="sup")
                    for q in range(q0, 4):
                        for p in range(2):
                            nc.tensor.matmul(
                                sup[:, q * 256 : (q + 1) * 256],
                                lt[:, 2 * p : 2 * p + 2, m * 128 : (m + 1) * 128],
                                rt[:, 2 * p : 2 * p + 2, q * 256 : (q + 1) * 256],
                                start=(p == 0), stop=(p == 1),
                                perf_mode=DR,
                            )
                    src = sup[:, lo:HS]
                    stagedp = (tile_counter * STAGE_NUM) // STAGE_DEN != (
                        (tile_counter + 1) * STAGE_NUM
                    ) // STAGE_DEN
                    tile_counter += 1
                    if stagedp:
                        st = staged.tile([128, HS], f16, tag="st", name="st")
                        nc.scalar.copy(st[:, :width], src)
                        r = run16[n_staged % NRUN]
                        n_staged += 1
                        nc.vector.tensor_tensor(
                            r[:, :width], r[:, :width], st[:, :width], MIN
                        )
                    else:
                        r = run32[n_direct % NRUN]
                        n_direct += 1
                        nc.vector.tensor_tensor(
                            r[:, :width], r[:, :width], src, MIN
                        )

            # tree-fold the running buffers, then one reduce
            nc.vector.tensor_tensor(run32[0][:], run32[0][:], run32[1][:], MIN)
            nc.vector.tensor_tensor(run16[0][:], run16[0][:], run16[1][:], MIN)
            nc.vector.tensor_tensor(run16[0][:], run16[0][:], run32[0][:], MIN)
            red = small.tile([128, 2 * NRUN], f32, tag="red", name="red")
            nc.vector.memset(red[:], 3.0e38)
            nc.vector.tensor_reduce(
                red[:, 0:1], run16[0][:], mybir.AxisListType.X, MIN
            )
            nc.sync.dma_start(out=min_out[:], in_=red[:])

    nc.finalize()
    return nc


def _core_rows(c: int):
    return [c, NHS - 1 - c]


def _core_items(c: int):
    i0, i1 = _core_rows(c)
    items = [(i0, i0), (i1, i1)]
    items += [(i0, j) for j in range(i0 + 1, NHS)]
    items += [(i1, j) for j in range(i1 + 1, NHS)]
    assert len(items) == NBLK
    return items


def _core_graphs(c: int):
    gph = HS // G  # 8 graphs per half-slab
    out = []
    for i in _core_rows(c):
        out.extend(range(i * gph, i * gph + gph))
    return out


def build_in_maps(emb: np.ndarray):
    import ml_dtypes

    emb = np.asarray(emb, dtype=np.float32)
    e8 = emb.astype(ml_dtypes.float8_e4m3)
    e16 = emb.astype(np.float16)

    # packed [p, t, col]: element = E[col, t*128 + p]
    p8 = np.ascontiguousarray(e8.T.reshape(KC, 128, N).transpose(1, 0, 2))
    p16 = np.ascontiguousarray(e16.T.reshape(KC, 128, N).transpose(1, 0, 2))
    slab8 = [
        np.ascontiguousarray(p8[:, :, s * HS : (s + 1) * HS]) for s in range(NHS)
    ]
    graph16 = [
        np.ascontiguousarray(p16[:, :, g * G : (g + 1) * G])
        for g in range(N_GRAPHS)
    ]

    in_maps = []
    for c in range(NCORES):
        items = _core_items(c)
        lhs = np.stack([slab8[i] for i, _ in items])
        rhs = np.stack([slab8[j] for _, j in items])
        dg = np.stack([graph16[g] for g in _core_graphs(c)])
        in_maps.append({"lhs8": lhs, "rhs8": rhs, "dg16": dg})
    return in_maps


def kernel(embeddings, row_idx, col_idx):
    from concourse.bass_utils import run_bass_kernel_spmd

    emb = np.asarray(embeddings, dtype=np.float32)
    row_idx = np.asarray(row_idx)
    col_idx = np.asarray(col_idx)

    if "nc" not in _CACHED:
        _CACHED["nc"] = _build_program()
    nc = _CACHED["nc"]

    in_maps = build_in_maps(emb)
    res = run_bass_kernel_spmd(nc, in_maps, list(range(NCORES)))

    m = min(float(r["min_out"].min()) for r in res.results)

    blocks = np.empty((N_GRAPHS, G, G), np.float32)
    for c in range(NCORES):
        for idx, g in enumerate(_core_graphs(c)):
            blocks[g] = res.results[c]["diag_out"][idx]

    # Gram max is attained on the diagonal (AM-GM), which the exact
    # diagonal blocks contain.
    M = float(blocks[:, np.arange(G), np.arange(G)].max())

    norm = (blocks - m) / (M - m + EPS)
    blk = row_idx // G
    out = norm[blk, row_idx % G, col_idx % G].astype(np.float32)
    return out


# revision 8
# speedup vs baseline: 2272.7582x; 1.0772x over previous
"""LinkPredictor similarity kernel v2 for 8 Trainium2 NeuronCores.

reference:
    sims = E @ E.T               # [16384, 16384], E = [16384, 512] fp32
    m, M = sims.min(), sims.max()
    sims = (sims - m) / (M - m + 1e-7)
    out  = sims[row_idx, col_idx]     # block-diag strict-upper-tri gather

Key ideas vs the v1 baseline (fp32r sweep + min&max scans, 269 us sim):
  * Gram max is always on the diagonal: s_ij <= max(s_ii, s_jj) since
    ||x_i - x_j||^2 >= 0. M comes free from the exact diagonal-graph-block
    pass, eliminating the max scan entirely.
  * The min sweep only needs each entry to ~+-2 absolute out of a ~840
    range, so fp8 e4m3 inputs suffice (measured end-to-end rel err 0.7%
    vs the 2e-2 gate). fp8 DoubleRow contracts 2 k-tiles per pass at 0.5
    cycles/row: 4x PE throughput in the cost model, ~2x on silicon.
  * Min eviction: each PSUM supertile becomes ONE DVE tensor_scalar op
    whose op1=min accumulator emits a [128,1] column min — 4x perf mode on
    ScalarE-staged fp16 tiles (19/34 of tiles), 1x direct from PSUM fp32
    for the rest. No running buffers, no dependency chains; a single
    tensor_reduce over the column matrix finishes the min.
  * Diagonal slab-blocks skip quarter-tiles strictly below the diagonal
    (mirrors of already-covered upper-triangle entries).
  * The diagonal-graph-block pass is interleaved into the sweep so its
    DMA/PE/ScalarE work hides under the min scan.

Distribution: 16 half-slabs of 1024 rows. Core c owns half-slabs
{c, 15-c} and the 17 upper-triangle [1024,1024] blocks whose row
half-slab is one of those. Items are ordered [diag, diag, 15 x offdiag]
so one SPMD program serves all cores. Host combines min/max, normalizes,
gathers.
"""

import numpy as np

N_GRAPHS = 128
G = 128
D = 512
N = N_GRAPHS * G          # 16384
EPS = 1e-7
NCORES = 8
HS = 1024                 # half-slab rows
NHS = N // HS             # 16 half-slabs
NBLK = 17                 # triangle blocks per core
KC = D // 128             # 4 contraction k-tiles
MT = HS // 128            # 8 m-strips per block
GPC = 16                  # graphs per core

# eviction: every PSUM supertile is reduced to one [128,1] column of `red`
# by a single DVE tensor_scalar op (op1=min drives its accumulator) — 4x
# perf mode for ScalarE-staged fp16 tiles, 1x direct from PSUM fp32.
STAGE_NUM, STAGE_DEN = 19, 34   # staged fraction of evictions

_CACHED = {}

# first needed column-quarter per m-strip inside a diagonal slab-block
_DIAG_Q0 = [0, 0, 1, 1, 2, 2, 3, 3]


def _build_program():
    import concourse.bacc as bacc
    import concourse.mybir as mybir
    from concourse.tile import TileContext

    f32 = mybir.dt.float32
    f16 = mybir.dt.float16
    f8 = mybir.dt.float8e4
    DR = mybir.MatmulPerfMode.DoubleRow
    MIN = mybir.AluOpType.min

    nc = bacc.Bacc(target_bir_lowering=False)
    lhs8 = nc.declare_dram_parameter("lhs8", [NBLK, 128, KC, HS], f8, isOutput=False)
    rhs8 = nc.declare_dram_parameter("rhs8", [NBLK, 128, KC, HS], f8, isOutput=False)
    dg16 = nc.declare_dram_parameter("dg16", [GPC, 128, KC, G], f16, isOutput=False)
    diag_out = nc.declare_dram_parameter("diag_out", [GPC, G, G], f32, isOutput=True)
    min_out = nc.declare_dram_parameter("min_out", [128, 2], f32, isOutput=True)

    with TileContext(nc) as tc:
        with (
            tc.tile_pool(name="persist", bufs=1) as persist,
            tc.tile_pool(name="stream", bufs=3) as stream,
            tc.tile_pool(name="staged", bufs=4) as staged,
            tc.tile_pool(name="scratch", bufs=4) as scratch,
            tc.tile_pool(name="small", bufs=4) as small,
            tc.tile_pool(name="ps", bufs=4, space="PSUM") as ps,
        ):
            red = persist.tile([128, 144], f32, tag="red", name="red")
            nc.vector.memset(red[:], 3.0e38)

            def emit_dg(g):
                dgt = small.tile([128, KC, G], f16, tag="dgt", name="dgt")
                nc.sync.dma_start(out=dgt[:], in_=dg16[g])
                dacc_t = ps.tile([128, HS], f32, tag="sup", name="dacc_sup")
                dacc = dacc_t[:, :G]
                for k in range(KC):
                    nc.tensor.matmul(
                        dacc, dgt[:, k, :], dgt[:, k, :],
                        start=(k == 0), stop=(k == KC - 1),
                    )
                dcp = small.tile([128, G], f32, tag="dcp", name="dcp")
                nc.scalar.copy(dcp[:], dacc)
                nc.sync.dma_start(out=diag_out[g], in_=dcp[:])

            tile_counter = 0
            col = 0
            for item in range(NBLK):
                if item < GPC:
                    emit_dg(item)
                is_diag = item < 2
                rt = stream.tile([128, KC, HS], f8, tag="rt", name="rt")
                nc.sync.dma_start(out=rt[:], in_=rhs8[item])
                if is_diag:
                    lt = rt
                else:
                    lt = stream.tile([128, KC, HS], f8, tag="lt", name="lt")
                    nc.sync.dma_start(out=lt[:], in_=lhs8[item])
                for m in range(MT):
                    q0 = _DIAG_Q0[m] if is_diag else 0
                    lo = m * 128 if is_diag else 0
                    width = HS - lo
                    sup = ps.tile([128, HS], f32, tag="sup", name="sup")
                    for q in range(q0, 4):
                        for p in range(2):
                            nc.tensor.matmul(
                                sup[:, q * 256 : (q + 1) * 256],
                                lt[:, 2 * p : 2 * p + 2, m * 128 : (m + 1) * 128],
                                rt[:, 2 * p : 2 * p + 2, q * 256 : (q + 1) * 256],
                                start=(p == 0), stop=(p == 1),
                                perf_mode=DR,
                            )
                    src = sup[:, lo:HS]
                    stagedp = (tile_counter * STAGE_NUM) // STAGE_DEN != (
                        (tile_counter + 1) * STAGE_NUM
                    ) // STAGE_DEN
                    tile_counter += 1
                    scr = scratch.tile([128, HS], f16, tag="scr", name="scr")
                    if stagedp:
                        st = staged.tile([128, HS], f16, tag="st", name="st")
                        nc.scalar.copy(st[:, :width], src)
                        nc.vector.tensor_scalar(
                            scr[:, :width], st[:, :width], 60000.0, None,
                            MIN, MIN, accum_out=red[:, col : col + 1],
                        )
                    else:
                        nc.vector.tensor_scalar(
                            scr[:, :width], src, 60000.0, None,
                            MIN, MIN, accum_out=red[:, col : col + 1],
                        )
                    col += 1

            redf = small.tile([128, 2], f32, tag="redf", name="redf")
            nc.vector.memset(redf[:], 3.0e38)
            nc.vector.tensor_reduce(
                redf[:, 0:1], red[:, :col], mybir.AxisListType.X, MIN
            )
            nc.sync.dma_start(out=min_out[:], in_=redf[:])

    nc.finalize()
    return nc


def _core_rows(c: int):
    return [c, NHS - 1 - c]


def _core_items(c: int):
    i0, i1 = _core_rows(c)
    items = [(i0, i0), (i1, i1)]
    items += [(i0, j) for j in range(i0 + 1, NHS)]
    items += [(i1, j) for j in range(i1 + 1, NHS)]
    assert len(items) == NBLK
    return items


def _core_graphs(c: int):
    gph = HS // G  # 8 graphs per half-slab
    out = []
    for i in _core_rows(c):
        out.extend(range(i * gph, i * gph + gph))
    return out


def build_in_maps(emb: np.ndarray):
    import ml_dtypes

    emb = np.asarray(emb, dtype=np.float32)
    e8 = emb.astype(ml_dtypes.float8_e4m3)
    e16 = emb.astype(np.float16)

    # packed [p, t, col]: element = E[col, t*128 + p]
    p8 = np.ascontiguousarray(e8.T.reshape(KC, 128, N).transpose(1, 0, 2))
    p16 = np.ascontiguousarray(e16.T.reshape(KC, 128, N).transpose(1, 0, 2))
    slab8 = [
        np.ascontiguousarray(p8[:, :, s * HS : (s + 1) * HS]) for s in range(NHS)
    ]
    graph16 = [
        np.ascontiguousarray(p16[:, :, g * G : (g + 1) * G])
        for g in range(N_GRAPHS)
    ]

    in_maps = []
    for c in range(NCORES):
        items = _core_items(c)
        lhs = np.stack([slab8[i] for i, _ in items])
        rhs = np.stack([slab8[j] for _, j in items])
        dg = np.stack([graph16[g] for g in _core_graphs(c)])
        in_maps.append({"lhs8": lhs, "rhs8": rhs, "dg16": dg})
    return in_maps


def kernel(embeddings, row_idx, col_idx):
    from concourse.bass_utils import run_bass_kernel_spmd

    emb = np.asarray(embeddings, dtype=np.float32)
    row_idx = np.asarray(row_idx)
    col_idx = np.asarray(col_idx)

    if "nc" not in _CACHED:
        _CACHED["nc"] = _build_program()
    nc = _CACHED["nc"]

    in_maps = build_in_maps(emb)
    res = run_bass_kernel_spmd(nc, in_maps, list(range(NCORES)))

    m = min(float(r["min_out"].min()) for r in res.results)

    blocks = np.empty((N_GRAPHS, G, G), np.float32)
    for c in range(NCORES):
        for idx, g in enumerate(_core_graphs(c)):
            blocks[g] = res.results[c]["diag_out"][idx]

    # Gram max is attained on the diagonal (AM-GM), which the exact
    # diagonal blocks contain.
    M = float(blocks[:, np.arange(G), np.arange(G)].max())

    norm = (blocks - m) / (M - m + EPS)
    blk = row_idx // G
    out = norm[blk, row_idx % G, col_idx % G].astype(np.float32)
    return out


# revision 9
# speedup vs baseline: 2359.2552x; 1.0381x over previous
"""LinkPredictor similarity kernel v2 for 8 Trainium2 NeuronCores.

reference:
    sims = E @ E.T               # [16384, 16384], E = [16384, 512] fp32
    m, M = sims.min(), sims.max()
    sims = (sims - m) / (M - m + 1e-7)
    out  = sims[row_idx, col_idx]     # block-diag strict-upper-tri gather

Key ideas vs the v1 baseline (fp32r sweep + min&max scans, 269 us sim):
  * Gram max is always on the diagonal: s_ij <= max(s_ii, s_jj) since
    ||x_i - x_j||^2 >= 0. M comes free from the exact diagonal-graph-block
    pass, eliminating the max scan entirely.
  * The min sweep only needs each entry to ~+-2 absolute out of a ~840
    range, so fp8 e4m3 inputs suffice (measured end-to-end rel err 0.7%
    vs the 2e-2 gate). fp8 DoubleRow contracts 2 k-tiles per pass at 0.5
    cycles/row: 4x PE throughput in the cost model, ~2x on silicon.
  * Min eviction: each PSUM supertile becomes ONE DVE tensor_scalar op
    whose op1=min accumulator emits a [128,1] column min — 4x perf mode on
    ScalarE-staged fp16 tiles (5/8 of tiles), 1x direct from PSUM fp32
    for the rest. No running buffers, no dependency chains; a single
    tensor_reduce over the column matrix finishes the min.
  * Diagonal slab-blocks skip quarter-tiles strictly below the diagonal
    (mirrors of already-covered upper-triangle entries).
  * The diagonal-graph-block pass is interleaved into the sweep so its
    DMA/PE/ScalarE work hides under the min scan.

Distribution: 16 half-slabs of 1024 rows. Core c owns half-slabs
{c, 15-c} and the 17 upper-triangle [1024,1024] blocks whose row
half-slab is one of those. Items are ordered [diag, diag, 15 x offdiag]
so one SPMD program serves all cores. Host combines min/max, normalizes,
gathers.
"""

import numpy as np

N_GRAPHS = 128
G = 128
D = 512
N = N_GRAPHS * G          # 16384
EPS = 1e-7
NCORES = 8
HS = 1024                 # half-slab rows
NHS = N // HS             # 16 half-slabs
NBLK = 17                 # triangle blocks per core
KC = D // 128             # 4 contraction k-tiles
MT = HS // 128            # 8 m-strips per block
GPC = 16                  # graphs per core

# eviction: every PSUM supertile is reduced to one [128,1] column of `red`
# by a single DVE tensor_scalar op (op1=min drives its accumulator) — 4x
# perf mode for ScalarE-staged fp16 tiles, 1x direct from PSUM fp32.
STAGE_NUM, STAGE_DEN = 5, 8     # staged fraction of evictions

_CACHED = {}

# first needed column-quarter per m-strip inside a diagonal slab-block
_DIAG_Q0 = [0, 0, 1, 1, 2, 2, 3, 3]


def _build_program():
    import concourse.bacc as bacc
    import concourse.mybir as mybir
    from concourse.tile import TileContext

    f32 = mybir.dt.float32
    f16 = mybir.dt.float16
    f8 = mybir.dt.float8e4
    DR = mybir.MatmulPerfMode.DoubleRow
    MIN = mybir.AluOpType.min

    nc = bacc.Bacc(target_bir_lowering=False)
    lhs8 = nc.declare_dram_parameter("lhs8", [NBLK, 128, KC, HS], f8, isOutput=False)
    rhs8 = nc.declare_dram_parameter("rhs8", [NBLK, 128, KC, HS], f8, isOutput=False)
    dg16 = nc.declare_dram_parameter("dg16", [GPC, 128, KC, G], f16, isOutput=False)
    diag_out = nc.declare_dram_parameter("diag_out", [GPC, G, G], f32, isOutput=True)
    min_out = nc.declare_dram_parameter("min_out", [128, 2], f32, isOutput=True)

    with TileContext(nc) as tc:
        with (
            tc.tile_pool(name="persist", bufs=1) as persist,
            tc.tile_pool(name="stream", bufs=3) as stream,
            tc.tile_pool(name="staged", bufs=4) as staged,
            tc.tile_pool(name="scratch", bufs=4) as scratch,
            tc.tile_pool(name="small", bufs=4) as small,
            tc.tile_pool(name="ps", bufs=4, space="PSUM") as ps,
        ):
            red = persist.tile([128, 144], f32, tag="red", name="red")
            nc.vector.memset(red[:], 3.0e38)

            def emit_dg(g):
                dgt = small.tile([128, KC, G], f16, tag="dgt", name="dgt")
                nc.sync.dma_start(out=dgt[:], in_=dg16[g])
                dacc_t = ps.tile([128, HS], f32, tag="sup", name="dacc_sup")
                dacc = dacc_t[:, :G]
                for k in range(KC):
                    nc.tensor.matmul(
                        dacc, dgt[:, k, :], dgt[:, k, :],
                        start=(k == 0), stop=(k == KC - 1),
                    )
                dcp = small.tile([128, G], f32, tag="dcp", name="dcp")
                nc.scalar.copy(dcp[:], dacc)
                nc.sync.dma_start(out=diag_out[g], in_=dcp[:])

            tile_counter = 0
            col = 0
            for item in range(NBLK):
                if item < GPC:
                    emit_dg(item)
                is_diag = item < 2
                rt = stream.tile([128, KC, HS], f8, tag="rt", name="rt")
                nc.sync.dma_start(out=rt[:], in_=rhs8[item])
                if is_diag:
                    lt = rt
                else:
                    lt = stream.tile([128, KC, HS], f8, tag="lt", name="lt")
                    nc.sync.dma_start(out=lt[:], in_=lhs8[item])
                for m in range(MT):
                    q0 = _DIAG_Q0[m] if is_diag else 0
                    lo = m * 128 if is_diag else 0
                    width = HS - lo
                    sup = ps.tile([128, HS], f32, tag="sup", name="sup")
                    for q in range(q0, 4):
                        for p in range(2):
                            nc.tensor.matmul(
                                sup[:, q * 256 : (q + 1) * 256],
                                lt[:, 2 * p : 2 * p + 2, m * 128 : (m + 1) * 128],
                                rt[:, 2 * p : 2 * p + 2, q * 256 : (q + 1) * 256],
                                start=(p == 0), stop=(p == 1),
                                perf_mode=DR,
                            )
                    src = sup[:, lo:HS]
                    stagedp = (tile_counter * STAGE_NUM) // STAGE_DEN != (
                        (tile_counter + 1) * STAGE_NUM
                    ) // STAGE_DEN
                    tile_counter += 1
                    scr = scratch.tile([128, HS], f16, tag="scr", name="scr")
                    if stagedp:
                        st = staged.tile([128, HS], f16, tag="st", name="st")
                        nc.scalar.copy(st[:, :width], src)
                        nc.vector.tensor_scalar(
                            scr[:, :width], st[:, :width], 60000.0, None,
                            MIN, MIN, accum_out=red[:, col : col + 1],
                        )
                    else:
                        nc.vector.tensor_scalar(
                            scr[:, :width], src, 60000.0, None,
                            MIN, MIN, accum_out=red[:, col : col + 1],
                        )
                    col += 1

            redf = small.tile([128, 2], f32, tag="redf", name="redf")
            nc.vector.memset(redf[:], 3.0e38)
            nc.vector.tensor_reduce(
                redf[:, 0:1], red[:, :col], mybir.AxisListType.X, MIN
            )
            nc.sync.dma_start(out=min_out[:], in_=redf[:])

    nc.finalize()
    return nc


def _core_rows(c: int):
    return [c, NHS - 1 - c]


def _core_items(c: int):
    i0, i1 = _core_rows(c)
    items = [(i0, i0), (i1, i1)]
    items += [(i0, j) for j in range(i0 + 1, NHS)]
    items += [(i1, j) for j in range(i1 + 1, NHS)]
    assert len(items) == NBLK
    return items


def _core_graphs(c: int):
    gph = HS // G  # 8 graphs per half-slab
    out = []
    for i in _core_rows(c):
        out.extend(range(i * gph, i * gph + gph))
    return out


def build_in_maps(emb: np.ndarray):
    import ml_dtypes

    emb = np.asarray(emb, dtype=np.float32)
    e8 = emb.astype(ml_dtypes.float8_e4m3)
    e16 = emb.astype(np.float16)

    # packed [p, t, col]: element = E[col, t*128 + p]
    p8 = np.ascontiguousarray(e8.T.reshape(KC, 128, N).transpose(1, 0, 2))
    p16 = np.ascontiguousarray(e16.T.reshape(KC, 128, N).transpose(1, 0, 2))
    slab8 = [
        np.ascontiguousarray(p8[:, :, s * HS : (s + 1) * HS]) for s in range(NHS)
    ]
    graph16 = [
        np.ascontiguousarray(p16[:, :, g * G : (g + 1) * G])
        for g in range(N_GRAPHS)
    ]

    in_maps = []
    for c in range(NCORES):
        items = _core_items(c)
        lhs = np.stack([slab8[i] for i, _ in items])
        rhs = np.stack([slab8[j] for _, j in items])
        dg = np.stack([graph16[g] for g in _core_graphs(c)])
        in_maps.append({"lhs8": lhs, "rhs8": rhs, "dg16": dg})
    return in_maps


def kernel(embeddings, row_idx, col_idx):
    from concourse.bass_utils import run_bass_kernel_spmd

    emb = np.asarray(embeddings, dtype=np.float32)
    row_idx = np.asarray(row_idx)
    col_idx = np.asarray(col_idx)

    if "nc" not in _CACHED:
        _CACHED["nc"] = _build_program()
    nc = _CACHED["nc"]

    in_maps = build_in_maps(emb)
    res = run_bass_kernel_spmd(nc, in_maps, list(range(NCORES)))

    m = min(float(r["min_out"].min()) for r in res.results)

    blocks = np.empty((N_GRAPHS, G, G), np.float32)
    for c in range(NCORES):
        for idx, g in enumerate(_core_graphs(c)):
            blocks[g] = res.results[c]["diag_out"][idx]

    # Gram max is attained on the diagonal (AM-GM), which the exact
    # diagonal blocks contain.
    M = float(blocks[:, np.arange(G), np.arange(G)].max())

    norm = (blocks - m) / (M - m + EPS)
    blk = row_idx // G
    out = norm[blk, row_idx % G, col_idx % G].astype(np.float32)
    return out


# revision 10
# speedup vs baseline: 2406.0854x; 1.0198x over previous
"""LinkPredictor similarity kernel v2 for 8 Trainium2 NeuronCores.

reference:
    sims = E @ E.T               # [16384, 16384], E = [16384, 512] fp32
    m, M = sims.min(), sims.max()
    sims = (sims - m) / (M - m + 1e-7)
    out  = sims[row_idx, col_idx]     # block-diag strict-upper-tri gather

Key ideas vs the v1 baseline (fp32r sweep + min&max scans, 269 us sim):
  * Gram max is always on the diagonal: s_ij <= max(s_ii, s_jj) since
    ||x_i - x_j||^2 >= 0. M comes free from the exact diagonal-graph-block
    pass, eliminating the max scan entirely.
  * The min sweep only needs each entry to ~+-2 absolute out of a ~840
    range, so fp8 e4m3 inputs suffice (measured end-to-end rel err 0.7%
    vs the 2e-2 gate). fp8 DoubleRow contracts 2 k-tiles per pass at 0.5
    cycles/row: 4x PE throughput in the cost model, ~2x on silicon.
  * Min eviction: each PSUM supertile becomes ONE DVE tensor_scalar op
    whose op1=min accumulator emits a [128,1] column min — 4x perf mode on
    ScalarE-staged fp16 tiles (5/8 of tiles), 1x direct from PSUM fp32
    for the rest. No running buffers, no dependency chains; a single
    tensor_reduce over the column matrix finishes the min.
  * Diagonal slab-blocks skip quarter-tiles strictly below the diagonal
    (mirrors of already-covered upper-triangle entries).
  * The diagonal-graph-block pass is interleaved into the sweep so its
    DMA/PE/ScalarE work hides under the min scan.

Distribution: 16 half-slabs of 1024 rows. Core c owns half-slabs
{c, 15-c} and the 17 upper-triangle [1024,1024] blocks whose row
half-slab is one of those. Items are ordered [diag, diag, 15 x offdiag]
so one SPMD program serves all cores. Host combines min/max, normalizes,
gathers.
"""

import numpy as np

N_GRAPHS = 128
G = 128
D = 512
N = N_GRAPHS * G          # 16384
EPS = 1e-7
NCORES = 8
HS = 1024                 # half-slab rows
NHS = N // HS             # 16 half-slabs
NBLK = 17                 # triangle blocks per core
KC = D // 128             # 4 contraction k-tiles
MT = HS // 128            # 8 m-strips per block
GPC = 16                  # graphs per core

# eviction: every PSUM supertile is reduced to one [128,1] column of `red`
# by a single DVE tensor_scalar op (op1=min drives its accumulator) — 4x
# perf mode for ScalarE-staged fp16 tiles, 1x direct from PSUM fp32.
STAGE_NUM, STAGE_DEN = 5, 8     # staged fraction of evictions

_CACHED = {}

# first needed column-quarter per m-strip inside a diagonal slab-block
_DIAG_Q0 = [0, 0, 1, 1, 2, 2, 3, 3]


def _build_program():
    import concourse.bacc as bacc
    import concourse.mybir as mybir
    from concourse.tile import TileContext

    f32 = mybir.dt.float32
    f16 = mybir.dt.float16
    f8 = mybir.dt.float8e4
    DR = mybir.MatmulPerfMode.DoubleRow
    MIN = mybir.AluOpType.min

    nc = bacc.Bacc(target_bir_lowering=False)
    lhs8 = nc.declare_dram_parameter("lhs8", [NBLK, 128, KC, HS], f8, isOutput=False)
    rhs8 = nc.declare_dram_parameter("rhs8", [NBLK, 128, KC, HS], f8, isOutput=False)
    dg16 = nc.declare_dram_parameter("dg16", [GPC, 128, KC, G], f16, isOutput=False)
    diag_out = nc.declare_dram_parameter("diag_out", [GPC, G, G], f32, isOutput=True)
    min_out = nc.declare_dram_parameter("min_out", [128, 2], f32, isOutput=True)

    with TileContext(nc) as tc:
        with (
            tc.tile_pool(name="persist", bufs=1) as persist,
            tc.tile_pool(name="stream", bufs=3) as stream,
            tc.tile_pool(name="staged", bufs=4) as staged,
            tc.tile_pool(name="scratch", bufs=4) as scratch,
            tc.tile_pool(name="small", bufs=4) as small,
            tc.tile_pool(name="ps", bufs=4, space="PSUM") as ps,
        ):
            red = persist.tile([128, 144], f32, tag="red", name="red")
            nc.vector.memset(red[:], 3.0e38)

            def emit_dg(g):
                dgt = small.tile([128, KC, G], f16, tag="dgt", name="dgt")
                nc.sync.dma_start(out=dgt[:], in_=dg16[g])
                dacc_t = ps.tile([128, HS], f32, tag="sup", name="dacc_sup")
                dacc = dacc_t[:, :G]
                for k in range(KC):
                    nc.tensor.matmul(
                        dacc, dgt[:, k, :], dgt[:, k, :],
                        start=(k == 0), stop=(k == KC - 1),
                    )
                dcp = small.tile([128, G], f32, tag="dcp", name="dcp")
                nc.scalar.copy(dcp[:], dacc)
                nc.sync.dma_start(out=diag_out[g], in_=dcp[:])

            # phase 7 of the 5/8 Bresenham = class pattern SDSDSSDS per
            # 8-strip block, the scheduling optimum found by pattern sweep
            tile_counter = 7
            col = 0
            for item in range(NBLK):
                if item < GPC:
                    emit_dg(item)
                is_diag = item < 2
                rt = stream.tile([128, KC, HS], f8, tag="rt", name="rt")
                nc.sync.dma_start(out=rt[:], in_=rhs8[item])
                if is_diag:
                    lt = rt
                else:
                    lt = stream.tile([128, KC, HS], f8, tag="lt", name="lt")
                    nc.sync.dma_start(out=lt[:], in_=lhs8[item])
                for m in range(MT):
                    q0 = _DIAG_Q0[m] if is_diag else 0
                    lo = m * 128 if is_diag else 0
                    width = HS - lo
                    sup = ps.tile([128, HS], f32, tag="sup", name="sup")
                    for q in range(q0, 4):
                        for p in range(2):
                            nc.tensor.matmul(
                                sup[:, q * 256 : (q + 1) * 256],
                                lt[:, 2 * p : 2 * p + 2, m * 128 : (m + 1) * 128],
                                rt[:, 2 * p : 2 * p + 2, q * 256 : (q + 1) * 256],
                                start=(p == 0), stop=(p == 1),
                                perf_mode=DR,
                            )
                    src = sup[:, lo:HS]
                    stagedp = (tile_counter * STAGE_NUM) // STAGE_DEN != (
                        (tile_counter + 1) * STAGE_NUM
                    ) // STAGE_DEN
                    tile_counter += 1
                    scr = scratch.tile([128, HS], f16, tag="scr", name="scr")
                    if stagedp:
                        st = staged.tile([128, HS], f16, tag="st", name="st")
                        nc.scalar.copy(st[:, :width], src)
                        nc.vector.tensor_scalar(
                            scr[:, :width], st[:, :width], 60000.0, None,
                            MIN, MIN, accum_out=red[:, col : col + 1],
                        )
                    else:
                        nc.vector.tensor_scalar(
                            scr[:, :width], src, 60000.0, None,
                            MIN, MIN, accum_out=red[:, col : col + 1],
                        )
                    col += 1

            redf = small.tile([128, 2], f32, tag="redf", name="redf")
            nc.vector.memset(redf[:], 3.0e38)
            nc.vector.tensor_reduce(
                redf[:, 0:1], red[:, :col], mybir.AxisListType.X, MIN
            )
            nc.sync.dma_start(out=min_out[:], in_=redf[:])

    nc.finalize()
    return nc


def _core_rows(c: int):
    return [c, NHS - 1 - c]


def _core_items(c: int):
    i0, i1 = _core_rows(c)
    items = [(i0, i0), (i1, i1)]
    items += [(i0, j) for j in range(i0 + 1, NHS)]
    items += [(i1, j) for j in range(i1 + 1, NHS)]
    assert len(items) == NBLK
    return items


def _core_graphs(c: int):
    gph = HS // G  # 8 graphs per half-slab
    out = []
    for i in _core_rows(c):
        out.extend(range(i * gph, i * gph + gph))
    return out


def build_in_maps(emb: np.ndarray):
    import ml_dtypes

    emb = np.asarray(emb, dtype=np.float32)
    e8 = emb.astype(ml_dtypes.float8_e4m3)
    e16 = emb.astype(np.float16)

    # packed [p, t, col]: element = E[col, t*128 + p]
    p8 = np.ascontiguousarray(e8.T.reshape(KC, 128, N).transpose(1, 0, 2))
    p16 = np.ascontiguousarray(e16.T.reshape(KC, 128, N).transpose(1, 0, 2))
    slab8 = [
        np.ascontiguousarray(p8[:, :, s * HS : (s + 1) * HS]) for s in range(NHS)
    ]
    graph16 = [
        np.ascontiguousarray(p16[:, :, g * G : (g + 1) * G])
        for g in range(N_GRAPHS)
    ]

    in_maps = []
    for c in range(NCORES):
        items = _core_items(c)
        lhs = np.stack([slab8[i] for i, _ in items])
        rhs = np.stack([slab8[j] for _, j in items])
        dg = np.stack([graph16[g] for g in _core_graphs(c)])
        in_maps.append({"lhs8": lhs, "rhs8": rhs, "dg16": dg})
    return in_maps


def kernel(embeddings, row_idx, col_idx):
    from concourse.bass_utils import run_bass_kernel_spmd

    emb = np.asarray(embeddings, dtype=np.float32)
    row_idx = np.asarray(row_idx)
    col_idx = np.asarray(col_idx)

    if "nc" not in _CACHED:
        _CACHED["nc"] = _build_program()
    nc = _CACHED["nc"]

    in_maps = build_in_maps(emb)
    res = run_bass_kernel_spmd(nc, in_maps, list(range(NCORES)))

    m = min(float(r["min_out"].min()) for r in res.results)

    blocks = np.empty((N_GRAPHS, G, G), np.float32)
    for c in range(NCORES):
        for idx, g in enumerate(_core_graphs(c)):
            blocks[g] = res.results[c]["diag_out"][idx]

    # Gram max is attained on the diagonal (AM-GM), which the exact
    # diagonal blocks contain.
    M = float(blocks[:, np.arange(G), np.arange(G)].max())

    norm = (blocks - m) / (M - m + EPS)
    blk = row_idx // G
    out = norm[blk, row_idx % G, col_idx % G].astype(np.float32)
    return out
